# revision 2
# baseline (speedup 1.0000x reference)
"""GRU cell kernel for Trainium2, 8-core data-parallel.

Strategy
--------
Data-parallel on batch across 8 cores; each core's shard is processed in
two host-level rounds of 1024 rows (same compiled NEFF dispatched twice).
All on-chip compute happens in *transposed space* ([hidden, batch]) so
every matmul contraction lands on SBUF partitions with no on-device
transposes:

    r^T = sigmoid(W_r @ x^T + U_r @ h^T + b_r)
    u^T = sigmoid(W_u @ x^T + U_u @ h^T + b_u)
    c^T = tanh   (W   @ x^T + U  @ (h.r)^T + b_c)
    o^T = h^T + u^T * (c^T - h^T)

Matmuls run in bf16 (4x the fp32 PE rate).  Weights + x/h shards are
fully SBUF-resident, so no DMA ever writes a recycled tile slot — this
toolchain's DMA descriptors encode exactly ONE sync wait, so any DMA
needing a cross-engine WAR/RAW wait on top of its queue-FIFO wait fails
walrus codegen.  Loads carry only queue waits; the 8 output stores go
out via 8 distinct SWDGE queues (no queue backpressure -> their single
RAW wait fits).  Biases ride the ScalarE activation (per-partition bias)
which also evicts PSUM and casts in the same instruction.
"""

import sys

sys.path.insert(0, "/opt/trn_rl_repo")

import numpy as np
import ml_dtypes
from contextlib import ExitStack

import concourse.bass as bass
import concourse.bacc as bacc
import concourse.mybir as mybir
from concourse import tile
from concourse.bass_utils import run_bass_kernel_spmd

BF16 = mybir.dt.bfloat16
F32 = mybir.dt.float32
AF = mybir.ActivationFunctionType

N_CORES = 8
B = 16384
D = 1024  # IN == H
N_ROUNDS = 2
B_SHARD = B // N_CORES // N_ROUNDS  # 1024 rows per core per round
BW = 512  # matmul moving width (one fp32 PSUM bank)


def build_nc(d=D, b_shard=B_SHARD, bw=BW):
    """Build the SPMD per-core Bass program.

    Packed weight order: 0=W_r, 1=U_r, 2=W_u, 3=U_u, 4=W, 5=U.
    Bias columns: [r: 0..nh) [u: nh..2nh) [c: 2nh..3nh).
    """
    nk = d // 128
    nh = d // 128
    nb = b_shard // bw

    nc = bacc.Bacc("TRN2", target_bir_lowering=False)
    xt = nc.dram_tensor("xt", [d, b_shard], BF16, kind="ExternalInput")
    ht = nc.dram_tensor("ht", [d, b_shard], BF16, kind="ExternalInput")
    wts = nc.dram_tensor("wts", [6, nh, nk, 128, 128], BF16, kind="ExternalInput")
    bias = nc.dram_tensor("bias", [128, 3 * nh], F32, kind="ExternalInput")
    out = nc.dram_tensor("out", [d, b_shard], F32, kind="ExternalOutput")

    with tile.TileContext(nc) as tc, ExitStack() as ctx:
        xp = ctx.enter_context(tc.tile_pool(name="xp", bufs=nk))
        hp = ctx.enter_context(tc.tile_pool(name="hp", bufs=nk))
        up = ctx.enter_context(tc.tile_pool(name="up", bufs=nh))
        hrp = ctx.enter_context(tc.tile_pool(name="hrp", bufs=nh))
        cp = ctx.enter_context(tc.tile_pool(name="cp", bufs=nh))
        rp = ctx.enter_context(tc.tile_pool(name="rp", bufs=2))
        # every weight tile gets its own slot: no DMA slot reuse anywhere
        wp = ctx.enter_context(tc.tile_pool(name="wp", bufs=6 * nh * nk))
        bp = ctx.enter_context(tc.tile_pool(name="bp", bufs=1))
        op = ctx.enter_context(tc.tile_pool(name="op", bufs=2))
        pp = ctx.enter_context(tc.tile_pool(name="pp", bufs=8, space="PSUM"))

        btile = bp.tile([128, 3 * nh], F32, name="btile")
        nc.sync.dma_start(btile, bias[:, :])

        xts, hts = [], []
        for k in range(nk):
            xtile = xp.tile([128, b_shard], BF16, name="xtile")
            nc.sync.dma_start(xtile, xt[k * 128 : (k + 1) * 128, :])
            xts.append(xtile)
        for k in range(nk):
            htile = hp.tile([128, b_shard], BF16, name="htile")
            nc.sync.dma_start(htile, ht[k * 128 : (k + 1) * 128, :])
            hts.append(htile)

        def gate_matmuls(j, mat_x, mov_x, mat_h, mov_h):
            """Accumulate x-part + h-part for gate tile j into nb PSUM banks."""
            ps = [pp.tile([128, bw], F32, name="ps") for _ in range(nb)]
            for mi, (mat, mov) in enumerate(((mat_x, mov_x), (mat_h, mov_h))):
                for k in range(nk):
                    lhsT = wp.tile([128, 128], BF16, name="wtile")
                    nc.sync.dma_start(lhsT, wts[mat, j, k, :, :])
                    for b in range(nb):
                        nc.tensor.matmul(
                            ps[b],
                            lhsT,
                            mov[k][:, b * bw : (b + 1) * bw],
                            start=(mi == 0 and k == 0),
                            stop=(mi == 1 and k == nk - 1),
                        )
            return ps

        # R phase: r = sigmoid(...); hr = h * r  (hr feeds the c matmuls)
        hrs = []
        for j in range(nh):
            ps = gate_matmuls(j, 0, xts, 1, hts)
            rtile = rp.tile([128, b_shard], BF16, name="rtile")
            for b in range(nb):
                nc.scalar.activation(
                    rtile[:, b * bw : (b + 1) * bw], ps[b], AF.Sigmoid,
                    bias=btile[:, j : j + 1],
                )
            hrtile = hrp.tile([128, b_shard], BF16, name="hrtile")
            nc.vector.tensor_mul(hrtile, hts[j], rtile)
            hrs.append(hrtile)

        # U phase
        us = []
        for j in range(nh):
            ps = gate_matmuls(j, 2, xts, 3, hts)
            util = up.tile([128, b_shard], BF16, name="utile")
            for b in range(nb):
                nc.scalar.activation(
                    util[:, b * bw : (b + 1) * bw], ps[b], AF.Sigmoid,
                    bias=btile[:, nh + j : nh + j + 1],
                )
            us.append(util)

        # C phase: x-part first so late-j hr can still be in flight
        cs = []
        for j in range(nh):
            ps = gate_matmuls(j, 4, xts, 5, hrs)
            ctile = cp.tile([128, b_shard], BF16, name="ctile")
            for b in range(nb):
                nc.scalar.activation(
                    ctile[:, b * bw : (b + 1) * bw], ps[b], AF.Tanh,
                    bias=btile[:, 2 * nh + j : 2 * nh + j + 1],
                )
            cs.append(ctile)

        # OUT phase: o = h + u*(c - h)   (h in bf16; ~1e-3 extra rel err)
        for j in range(nh):
            t = op.tile([128, b_shard], F32, name="ttile")
            nc.vector.tensor_sub(t, cs[j], hts[j])
            nc.vector.tensor_mul(t, us[j], t)
            nc.vector.tensor_add(t, t, hts[j])
            # SWDGE: 8 stores over 8 SW queues -> no queue backpressure
            # wait, so the single RAW wait fits the descriptor.
            nc.gpsimd.dma_start(out[j * 128 : (j + 1) * 128, :], t)

    # Bacc lowering: splits multi-wait sync into InstEventSemaphore ops
    # (hardware allows one wait per instruction), allocates registers, etc.
    nc.compile()
    return nc


def pack_inputs(inputs, d=D, b_shard=B_SHARD, n_shards=N_CORES * N_ROUNDS):
    """Host-side shard + transpose + cast. Returns per-shard input maps."""
    nk = d // 128
    nh = d // 128
    x = np.asarray(inputs["x_t"], np.float32)
    h = np.asarray(inputs["h_prev"], np.float32)

    mats = [inputs["W_r"], inputs["U_r"], inputs["W_u"], inputs["U_u"],
            inputs["W"], inputs["U"]]
    wts = np.empty((6, nh, nk, 128, 128), ml_dtypes.bfloat16)
    for i, m in enumerate(mats):
        mt = np.asarray(m, np.float32).T.astype(ml_dtypes.bfloat16)  # [in, out]
        # wts[i, j, k][p, m] = M.T[k*128+p, j*128+m]
        wts[i] = mt.reshape(nk, 128, nh, 128).transpose(2, 0, 1, 3)

    b_r = np.asarray(inputs["b_Wr"], np.float32) + np.asarray(inputs["b_Ur"], np.float32)
    b_u = np.asarray(inputs["b_Wu"], np.float32) + np.asarray(inputs["b_Uu"], np.float32)
    b_c = np.asarray(inputs["b_W"], np.float32) + np.asarray(inputs["b_U"], np.float32)
    bias = np.concatenate(
        [bb.reshape(nh, 128).T for bb in (b_r, b_u, b_c)], axis=1
    ).astype(np.float32)  # [128, 3*nh]

    in_maps = []
    for s in range(n_shards):
        rows = slice(s * b_shard, (s + 1) * b_shard)
        xT = np.ascontiguousarray(x[rows].T).astype(ml_dtypes.bfloat16)
        hT = np.ascontiguousarray(h[rows].T).astype(ml_dtypes.bfloat16)
        in_maps.append({"xt": xT, "ht": hT, "wts": wts, "bias": bias})
    return in_maps


_NC_CACHE = {}


def _get_nc():
    if "nc" not in _NC_CACHE:
        _NC_CACHE["nc"] = build_nc()
    return _NC_CACHE["nc"]


def _run(inputs, **spmd_kwargs):
    nc = _get_nc()
    in_maps = pack_inputs(inputs)
    # shard s = core (s % 8), round (s // 8): round-major dispatch
    out = np.empty((B, D), np.float32)
    results = []
    for r in range(N_ROUNDS):
        maps_r = [in_maps[c * N_ROUNDS + r] for c in range(N_CORES)]
        kw = dict(spmd_kwargs)
        if kw.get("tmpdir"):
            kw["tmpdir"] = f"{kw['tmpdir']}/round{r}"
            import os

            os.makedirs(kw["tmpdir"], exist_ok=True)
        res = run_bass_kernel_spmd(nc, maps_r, list(range(N_CORES)), **kw)
        results.append(res)
        for c in range(N_CORES):
            s = c * N_ROUNDS + r
            out[s * B_SHARD : (s + 1) * B_SHARD, :] = res.results[c]["out"].T
    return out, results


def kernel(**inputs):
    out, _ = _run(inputs)
    return out



# revision 6
# speedup vs baseline: 1.5638x; 1.5638x over previous
"""GRU cell kernel for Trainium2, 8-core data-parallel, single dispatch.

Strategy
--------
Data-parallel on batch across 8 cores; each core handles 2048 rows in ONE
NEFF dispatch, processed as two in-kernel chunks of 1024 columns (SBUF
capacity).  All on-chip compute is in transposed space ([hidden, batch])
so matmul contractions land on SBUF partitions with no on-device
transposes:

    r^T = sigmoid(W_r @ x^T + U_r @ h^T + b_r)
    u^T = sigmoid(W_u @ x^T + U_u @ h^T + b_u)
    c^T = tanh   (W   @ x^T + U  @ (h.r)^T + b_c)
    o^T = h^T + u^T * (c^T - h^T)

Matmuls run in bf16.  v1 profiling showed the kernel was gated by load
DMAs: 409 per-tile descriptors (32-256KB each) serialized on the Sync
engine's single HWDGE ring at ~600ns/descriptor, leaving LDWEIGHTS
waiting 0.5-0.8us on weight arrival between every matmul pair (340ns
observed MM spacing vs the 226ns streaming floor).  Sub-64KB DMAs are
descriptor-dominated (~52GB/s); >=1MB DMAs hit 341-425GB/s.

So v2 host-prepacks everything into big [128, F] blocks and loads with a
handful of ~2MB DMAs split across BOTH HWDGE rings (weights on the
qAct ring via nc.scalar, x/h on the qSP ring via nc.sync) so weight
prefetch never queues behind input streaming.  All tiles are fully
SBUF-resident with no DMA-written slot ever recycled (this toolchain's
DMA descriptors carry exactly ONE sync wait, so a load needing a WAR
wait on top of its queue wait fails codegen).  Stores ride SWDGE
(gpsimd) queues with their single RAW wait.

The R phase is software-pipelined (x-parts of j+2 interleave with
h-parts of j) so the PE has x-side work while U_r/h^T are still in
flight at kernel start.  U/C/OUT are fused per j: u_j and c_j feed the
output combine immediately, so only r/u/c transients of 2 tiles each are
live and everything fits in ~196KB/partition of SBUF.
"""

import sys

sys.path.insert(0, "/opt/trn_rl_repo")

import numpy as np
import ml_dtypes
from contextlib import ExitStack

import concourse.bass as bass
import concourse.bacc as bacc
import concourse.mybir as mybir
from concourse import tile
from concourse.bass_utils import run_bass_kernel_spmd

BF16 = mybir.dt.bfloat16
F32 = mybir.dt.float32
AF = mybir.ActivationFunctionType

N_CORES = 8
B = 16384
D = 1024  # IN == H
B_CORE = B // N_CORES  # 2048 rows per core
N_CHUNKS = 2
CW = B_CORE // N_CHUNKS  # 1024 batch columns per chunk
BW = 512  # matmul moving width (one fp32 PSUM bank)
NH = D // 128  # 8 hidden row-tiles
NK = D // 128  # 8 contraction tiles
NB = CW // BW  # 2 PSUM banks per chunk row


def build_nc():
    """Build the SPMD per-core Bass program.

    DRAM inputs (all host-prepacked, partition-major):
      wts  [6, 128, NH*NK*128] bf16 : per mat m, wts[m][p, (j*NK+k)*128+c]
                                      = M_m.T[k*128+p, j*128+c]
                                      (mats: 0=W_r 1=U_r 2=W_u 3=U_u 4=W 5=U)
      xt   [N_CHUNKS, 128, NK*CW]   : xt[ch][p, k*CW+c] = x.T[k*128+p, ch*CW+c]
      ht   [N_CHUNKS, 128, NK*CW]   : same for h_prev
      bias [128, 3*NH] f32          : bias[p, g*NH+j] = b_g[j*128+p]
    Output:
      out  [D, B_CORE] f32          : out[d, b] = o.T[d, b]
    """
    nc = bacc.Bacc("TRN2", target_bir_lowering=False)
    wts = nc.dram_tensor("wts", [6, 128, NH * NK * 128], BF16, kind="ExternalInput")
    xt = nc.dram_tensor("xt", [N_CHUNKS, 128, NK * CW], BF16, kind="ExternalInput")
    ht = nc.dram_tensor("ht", [N_CHUNKS, 128, NK * CW], BF16, kind="ExternalInput")
    bias = nc.dram_tensor("bias", [128, 3 * NH], F32, kind="ExternalInput")
    out = nc.dram_tensor("out", [D, B_CORE], F32, kind="ExternalOutput")

    with tile.TileContext(nc) as tc, ExitStack() as ctx:
        wp = ctx.enter_context(tc.tile_pool(name="wp", bufs=6))
        xp = ctx.enter_context(tc.tile_pool(name="xp", bufs=N_CHUNKS))
        hp = ctx.enter_context(tc.tile_pool(name="hp", bufs=N_CHUNKS))
        hrp = ctx.enter_context(tc.tile_pool(name="hrp", bufs=NH))
        rp = ctx.enter_context(tc.tile_pool(name="rp", bufs=2))
        up = ctx.enter_context(tc.tile_pool(name="up", bufs=2))
        cp = ctx.enter_context(tc.tile_pool(name="cp", bufs=2))
        op = ctx.enter_context(tc.tile_pool(name="op", bufs=2))
        bp = ctx.enter_context(tc.tile_pool(name="bp", bufs=1))
        pp = ctx.enter_context(tc.tile_pool(name="pp", bufs=8, space="PSUM"))

        # ---- loads: few big DMAs, split across the two HWDGE rings ----
        # qAct ring (scalar): the 6 weight blocks, in first-use order.
        wtiles = []
        for m in range(6):
            wt = wp.tile([128, NH * NK * 128], BF16, name="wtile")
            nc.scalar.dma_start(wt, wts[m, :, :])
            wtiles.append(wt)
        # qSP ring (sync): bias + per-chunk x/h.
        btile = bp.tile([128, 3 * NH], F32, name="btile")
        nc.sync.dma_start(btile, bias[:, :])
        xts, hts = [], []
        for ch in range(N_CHUNKS):
            xtile = xp.tile([128, NK * CW], BF16, name="xtile")
            nc.sync.dma_start(xtile, xt[ch, :, :])
            htile = hp.tile([128, NK * CW], BF16, name="htile")
            nc.sync.dma_start(htile, ht[ch, :, :])
            xts.append(xtile)
            hts.append(htile)

        def w_ap(m, j, k):
            return wtiles[m][:, (j * NK + k) * 128 : (j * NK + k + 1) * 128]

        def mm_half(ps, m, j, mov, start, stop):
            """One mat's contraction into NB psum banks (bank innermost)."""
            for k in range(NK):
                lhsT = w_ap(m, j, k)
                for b in range(NB):
                    nc.tensor.matmul(
                        ps[b],
                        lhsT,
                        mov[:, k * CW + b * BW : k * CW + (b + 1) * BW],
                        start=(start and k == 0),
                        stop=(stop and k == NK - 1),
                    )

        def mm_half_t(ps, m, j, movs, start, stop):
            """Same but moving operand is per-k tiles (hr)."""
            for k in range(NK):
                lhsT = w_ap(m, j, k)
                for b in range(NB):
                    nc.tensor.matmul(
                        ps[b],
                        lhsT,
                        movs[k][:, b * BW : (b + 1) * BW],
                        start=(start and k == 0),
                        stop=(stop and k == NK - 1),
                    )

        def activate(dst, ps, fn, bcol):
            for b in range(NB):
                nc.scalar.activation(
                    dst[:, b * BW : (b + 1) * BW], ps[b], fn,
                    bias=btile[:, bcol : bcol + 1],
                )

        for ch in range(N_CHUNKS):
            xc, hc = xts[ch], hts[ch]

            # ---- R phase, software-pipelined depth 2: the x-parts of
            # j+1/j+2 give the PE work while h^T (and U_r) are still in
            # flight at kernel start.  Live PSUM groups <= 3 (6 banks).
            hrs = []

            def finish_r(j, ps):
                mm_half(ps, 1, j, hc, start=False, stop=True)
                rtile = rp.tile([128, CW], BF16, name="rtile")
                activate(rtile, ps, AF.Sigmoid, j)
                hrtile = hrp.tile([128, CW], BF16, name="hrtile")
                nc.vector.tensor_mul(
                    hrtile, hc[:, j * CW : (j + 1) * CW], rtile
                )
                hrs.append(hrtile)

            ps_list = []
            for j in range(NH):
                ps = [pp.tile([128, BW], F32, name="ps") for _ in range(NB)]
                mm_half(ps, 0, j, xc, start=True, stop=False)
                ps_list.append(ps)
                if j >= 2:
                    finish_r(j - 2, ps_list[j - 2])
            finish_r(NH - 2, ps_list[NH - 2])
            finish_r(NH - 1, ps_list[NH - 1])

            # ---- U + C + OUT fused per j ----
            for j in range(NH):
                psu = [pp.tile([128, BW], F32, name="ps") for _ in range(NB)]
                mm_half(psu, 2, j, xc, start=True, stop=False)
                mm_half(psu, 3, j, hc, start=False, stop=True)
                util = up.tile([128, CW], BF16, name="utile")
                activate(util, psu, AF.Sigmoid, NH + j)

                psc = [pp.tile([128, BW], F32, name="ps") for _ in range(NB)]
                mm_half(psc, 4, j, xc, start=True, stop=False)
                mm_half_t(psc, 5, j, hrs, start=False, stop=True)
                ctile = cp.tile([128, CW], BF16, name="ctile")
                activate(ctile, psc, AF.Tanh, 2 * NH + j)

                hj = hc[:, j * CW : (j + 1) * CW]
                t = op.tile([128, CW], F32, name="ttile")
                nc.vector.tensor_sub(t, ctile, hj)
                nc.vector.tensor_mul(t, util, t)
                nc.vector.tensor_add(t, t, hj)
                nc.gpsimd.dma_start(
                    out[j * 128 : (j + 1) * 128, ch * CW : (ch + 1) * CW], t
                )

    nc.compile()
    return nc


def pack_inputs(inputs):
    """Host-side shard + transpose + pack + cast. Per-core input maps."""
    x = np.asarray(inputs["x_t"], np.float32)
    h = np.asarray(inputs["h_prev"], np.float32)

    mats = [inputs["W_r"], inputs["U_r"], inputs["W_u"], inputs["U_u"],
            inputs["W"], inputs["U"]]
    wts = np.empty((6, 128, NH * NK * 128), ml_dtypes.bfloat16)
    for i, m in enumerate(mats):
        mt = np.asarray(m, np.float32).T.astype(ml_dtypes.bfloat16)  # [in, out]
        # [k,p,j,c] -> [p,j,k,c]
        wts[i] = (
            mt.reshape(NK, 128, NH, 128)
            .transpose(1, 2, 0, 3)
            .reshape(128, NH * NK * 128)
        )

    b_r = np.asarray(inputs["b_Wr"], np.float32) + np.asarray(inputs["b_Ur"], np.float32)
    b_u = np.asarray(inputs["b_Wu"], np.float32) + np.asarray(inputs["b_Uu"], np.float32)
    b_c = np.asarray(inputs["b_W"], np.float32) + np.asarray(inputs["b_U"], np.float32)
    bias = np.concatenate(
        [bb.reshape(NH, 128).T for bb in (b_r, b_u, b_c)], axis=1
    ).astype(np.float32)  # [128, 3*NH]

    def pack_bt(a_rows):  # [B_CORE, D] f32 -> [N_CHUNKS, 128, NK*CW] bf16
        at = np.ascontiguousarray(a_rows.T).astype(ml_dtypes.bfloat16)  # [D, B_CORE]
        o = np.empty((N_CHUNKS, 128, NK * CW), ml_dtypes.bfloat16)
        for ch in range(N_CHUNKS):
            o[ch] = (
                at[:, ch * CW : (ch + 1) * CW]
                .reshape(NK, 128, CW)
                .transpose(1, 0, 2)
                .reshape(128, NK * CW)
            )
        return o

    in_maps = []
    for c in range(N_CORES):
        rows = slice(c * B_CORE, (c + 1) * B_CORE)
        in_maps.append(
            {
                "xt": pack_bt(x[rows]),
                "ht": pack_bt(h[rows]),
                "wts": wts,
                "bias": bias,
            }
        )
    return in_maps


_NC_CACHE = {}


def _get_nc():
    if "nc" not in _NC_CACHE:
        _NC_CACHE["nc"] = build_nc()
    return _NC_CACHE["nc"]


def _run(inputs, **spmd_kwargs):
    nc = _get_nc()
    in_maps = pack_inputs(inputs)
    res = run_bass_kernel_spmd(nc, in_maps, list(range(N_CORES)), **spmd_kwargs)
    out = np.empty((B, D), np.float32)
    for c in range(N_CORES):
        out[c * B_CORE : (c + 1) * B_CORE, :] = res.results[c]["out"].T
    return out, [res]


def kernel(**inputs):
    out, _ = _run(inputs)
    return out
